# revision 60
# baseline (speedup 1.0000x reference)
"""Top-2 MoE (B=2, S=1024, D=1024, E=16, H=2048) on 8 Trainium2 NeuronCores.

Strategy (expert parallelism, per the sharding hint):
  - Launch A (device): token-sharded router. Each core computes logits for
    its T/8 tokens directly in [token, expert] layout (the x-shard k-tile is
    the stationary operand, Wr streams its 16 columns) as fp32r one-pass
    matmuls — fp22 products keep top-2 selection faithful (validated ~1e-7
    effect) at 1/4 the fp32 PE cost. Top-2 via the DVE max8 instruction,
    then the dense combine matrix comb[t, e] (normalized top-2 softmax
    weights, 0 elsewhere) is written out. Critical path uses two coarse DMAs
    (each dma_start costs ~0.7us of sequencer issue time).
  - Host: all-to-all "dispatch" — pure data movement. Tokens are gathered
    per expert (fixed capacity C per expert) and laid out feature-major;
    expert weights are re-tiled so every device DMA is a contiguous block.
  - Launch B (device): expert shards. Core c owns experts 2c, 2c+1 and runs
    the 2-layer exact-GELU MLP on its gathered tokens in [feature, token]
    layout, so W1/W2 load directly as the matmul stationary operand with no
    transposes. fc1 runs in fp8e4m3 with DoubleRow (two k-tiles per matmul,
    halved W1 traffic; end-to-end rel err 1.75e-2, sim-validated against
    the erf-GELU reference); fc2 stays bf16. All big loads ride the sync
    HWDGE ring in exact PE consumption order (the ring drains FIFO, so
    later loads can never starve the running layer; the scalar ring is
    round-robin-starved to ~1/4 bandwidth and only carries the head loads
    plus tiny biases). Phase order fc1(e0), fc1(e1)-g0, fc2(e0),
    fc1(e1) rest, fc2(e1): the hoisted group fills the conserved wire
    deficit at the fc1->fc2 boundary. The combine weight and fc2 bias are
    applied on device in one fused DVE op; outputs return bf16.
  - Host: all-to-all "combine" — the unshard-reduce. The residual stream
    starts from x on the token's home shard and each token's two expert
    slots are scatter-added into it.

If any expert overflows the capacity C (cannot happen for the reference
routing, which peaks at 282 tokens/expert), a bit-exact numpy fallback
computes the full layer instead.

Both launches warm the PE with dummy matmuls on a memset scratch during
the ~7us Tile preamble + DMA-engine ramp, so the real matmuls start at
2.4 GHz instead of paying the ~3.4us HAM cold window.

Measured on the dev harness: router ~21us + experts ~66-68us = ~87.5-88us
under quiet conditions (baseline as staged: 130us in the same
environment). Shared-HBM contention adds up to +3-7us in bad windows.
"""

import numpy as np

import concourse.bacc as bacc
import concourse.mybir as mybir
from concourse.tile import TileContext
from concourse import bass_utils

F32 = mybir.dt.float32
F32R = mybir.dt.float32r
BF16 = mybir.dt.bfloat16
AF = mybir.ActivationFunctionType
ALU = mybir.AluOpType

USE_BF16 = True  # expert-MLP matmul operand dtype (bf16 vs float32r)
FP8_FC1 = True   # fc1 in fp8e4m3 + DoubleRow (2 k-tiles/matmul); fc2 stays bf16
F8 = mybir.dt.float8e4

B, S, D, E, H = 2, 1024, 1024, 16, 2048
T = B * S
TOP_K = 2
TEMP = 1.0
NCORES = 8
EPC = E // NCORES          # experts per core
TPC = T // NCORES          # router tokens per core
C = 288                    # per-expert token capacity (observed max ~282)
P = 128

_progs = {}


def _build_router():
    """Token-major router: logits land directly as [token, expert] tiles
    (x-shard tile is the stationary operand, Wr streams its 16 columns),
    so no PE transpose / identity is needed and the whole critical path
    (Wr, then per-k x chunks) rides the sync HWDGE ring, which starts
    ~6 us earlier than the scalar ring."""
    nc = bacc.Bacc("TRN2", target_bir_lowering=False, debug=False,
                   num_devices=NCORES)
    TB = TPC // P  # token blocks
    xsT = nc.dram_tensor("xsT", [TB, D, P], F32R, kind="ExternalInput").ap()
    wr = nc.dram_tensor("wr", [D, E], F32R, kind="ExternalInput").ap()
    brc = nc.dram_tensor("brc", [P, E], F32, kind="ExternalInput").ap()
    comb = nc.dram_tensor("comb", [TPC, E], F32, kind="ExternalOutput").ap()

    KT = D // P  # 8 contraction tiles
    with TileContext(nc) as tc:
        with (
            tc.tile_pool(name="const", bufs=1) as const,
            tc.tile_pool(name="sb", bufs=2) as sb,
            tc.tile_pool(name="ps", bufs=2, space="PSUM") as psp,
        ):
            # PE warmup: the real MM phase is only ~2-3us, far shorter than
            # the ~3.4us HAM window, so it would run entirely at 1.2 GHz.
            # Dummy matmuls on a memset scratch fill the preamble/DMA-wait
            # window and un-throttle the PE before the real matmuls start.
            scr = const.tile([P, P], BF16, tag="warm")
            nc.vector.memset(scr, 0.0)
            pw = psp.tile([P, P], F32, tag="warm_ps", bufs=1)
            for _w in range(18):
                nc.tensor.matmul(pw, lhsT=scr, rhs=scr, start=True, stop=True)

            # HWDGE dma_start costs ~0.7 us of sequencer issue time, so the
            # critical path uses as few DMAs as possible, split across the
            # scalar (body starts ~5.8us) and sync (~6.8us) sequencers.
            wr_sb = const.tile([P, KT, E], F32R, tag="wr")
            nc.scalar.dma_start(out=wr_sb,
                                in_=wr.rearrange("(k p) e -> p k e", p=P))
            br_sb = const.tile([P, E], F32, tag="br")
            nc.scalar.dma_start(out=br_sb, in_=brc)

            # tb-major layout (host-prepared): each token block's x rides its
            # own ring, so block 0's matmuls + softmax chain pipeline under
            # block 1's load + matmuls instead of everything waiting for the
            # full x shard
            xs = sb.tile([P, TB, KT, P], F32R, tag="xs")
            for tb in range(TB):
                (nc.sync if tb == 0 else nc.scalar).dma_start(
                    out=xs[:, tb], in_=xsT[tb].rearrange("(k p) t -> p k t", p=P))

            for tb in range(TB):
                ps = psp.tile([P, E], F32, tag="lg")
                for k in range(KT):
                    nc.tensor.matmul(ps, lhsT=xs[:, tb, k, :],
                                     rhs=wr_sb[:, k, :],
                                     start=(k == 0), stop=(k == KT - 1))
                lg = sb.tile([P, E], F32, tag="lg_sb")
                nc.vector.tensor_tensor(lg, ps, br_sb, ALU.add)
                mx = sb.tile([P, 8], F32, tag="mx")
                nc.vector.max(mx, lg)
                negm1 = sb.tile([P, 1], F32, tag="negm1")
                nc.vector.tensor_scalar_mul(negm1, mx[:, 0:1], -1.0 / TEMP)
                s = sb.tile([P, E], F32, tag="s")
                nc.scalar.activation(s, lg, AF.Exp, bias=negm1, scale=1.0 / TEMP)
                e2 = sb.tile([P, 1], F32, tag="e2")
                nc.scalar.activation(e2, mx[:, 1:2], AF.Exp, bias=negm1,
                                     scale=1.0 / TEMP)
                den = sb.tile([P, 1], F32, tag="den")
                nc.vector.tensor_scalar_add(den, e2, 1.0)
                rec = sb.tile([P, 1], F32, tag="rec")
                nc.vector.reciprocal(rec, den)
                mask = sb.tile([P, E], F32, tag="mask")
                nc.vector.tensor_tensor(mask, lg, mx[:, 1:2].to_broadcast([P, E]),
                                        ALU.is_ge)
                cmb = sb.tile([P, E], F32, tag="cmb")
                nc.vector.scalar_tensor_tensor(cmb, s, rec, mask,
                                               ALU.mult, ALU.mult)
                (nc.scalar if tb == 0 else nc.sync).dma_start(
                    out=comb[tb * P:(tb + 1) * P, :], in_=cmb)
    nc.compile()
    return nc


def _build_experts(act=AF.Gelu, bf16=USE_BF16):
    assert EPC == 2, "phase schedule below is written for 2 experts/core"
    nc = bacc.Bacc("TRN2", target_bir_lowering=False, debug=False,
                   num_devices=NCORES)
    MT1 = H // P   # 16 fc1 output tiles
    KT1 = D // P   # 8 fc1 contraction tiles
    MT2 = D // P   # 8 fc2 output tiles
    KT2 = H // P   # 16 fc2 contraction tiles
    MM = BF16 if bf16 else F32R
    M1 = F8 if FP8_FC1 else MM  # fc1 operand dtype

    # weights pre-tiled on host, two output tiles per DMA (>=1 MiB transfers)
    w1l = nc.dram_tensor("w1l", [EPC, MT1 // 2, P, 2 * KT1, P], M1,
                         kind="ExternalInput").ap()
    w2l = nc.dram_tensor("w2l", [EPC, MT2 // 2, P, 2 * KT2, P], MM,
                         kind="ExternalInput").ap()
    xgm = nc.dram_tensor("xgm", [EPC, P, KT1, C], M1,
                         kind="ExternalInput").ap()
    b1t = nc.dram_tensor("b1t", [EPC, P, MT1], F32, kind="ExternalInput").ap()
    b2t = nc.dram_tensor("b2t", [EPC, P, MT2], F32, kind="ExternalInput").ap()
    wtr = nc.dram_tensor("wtr", [P, EPC, C], F32, kind="ExternalInput").ap()
    # combined output in bf16: it is scaled by the (<=1) combine weight and
    # added to the fp32 residual on host, so bf16 rounding here is ~1e-3
    ot = nc.dram_tensor("ot", [EPC, MT2, P, C], BF16, kind="ExternalOutput").ap()

    with TileContext(nc) as tc:
        with (
            tc.tile_pool(name="xg", bufs=2) as xgp,
            tc.tile_pool(name="wt", bufs=8) as wtp,
            tc.tile_pool(name="h", bufs=2 * MT1) as hp,
            tc.tile_pool(name="o", bufs=6) as op_,
            tc.tile_pool(name="small", bufs=2) as smp,
            tc.tile_pool(name="const", bufs=1) as cst,
            tc.tile_pool(name="ps", bufs=7, space="PSUM") as psp,
        ):
            # Critical path to the first MM group (expert 0, m-tile 0): one
            # coarse DMA per ring (each dma_start costs ~0.7us of sequencer
            # issue time, so fewer-but-bigger wins), issued first on the two
            # HWDGE sequencers.
            # PE warmup (see router): fills the ~7-11.5us preamble/DMA-wait
            # window so the first real matmuls run at 2.4 GHz
            scr = cst.tile([P, P], BF16, tag="warm")
            nc.vector.memset(scr, 0.0)
            pw = psp.tile([P, P], F32, tag="warm_ps", bufs=1)
            for _w in range(34):
                nc.tensor.matmul(pw, lhsT=scr, rhs=scr, start=True, stop=True)

            xg0 = xgp.tile([P, KT1, C], M1, tag="xg")
            w1f = wtp.tile([P, 2 * KT1, P], M1, tag="w1")
            nc.sync.dma_start(out=xg0, in_=xgm[0])
            nc.scalar.dma_start(out=w1f, in_=w1l[0, 0])
            b1f = smp.tile([P, MT1], F32, tag="b1")
            nc.scalar.dma_start(out=b1f, in_=b1t[0])

            wts_sb = cst.tile([P, EPC, C], F32, tag="wts")
            nc.gpsimd.dma_start(out=wts_sb, in_=wtr)

            # expert-1 activations/biases are issued from inside expert 0's
            # fc1 loop; fc2 biases ride the gpsimd ring
            xgs, b1s_, b2s_ = {0: xg0}, {0: b1f}, {}
            for e in range(EPC):
                b2s = smp.tile([P, MT2], F32, tag="b2")
                nc.gpsimd.dma_start(out=b2s, in_=b2t[e])
                b2s_[e] = b2s

            # All weight loads ride the sync HWDGE ring, issued in exact PE
            # consumption order — the ring drains FIFO at full width, so
            # later loads can never starve the layer currently running.
            # w2[e] blocks are slotted one-ahead: two late in fc1(e), two
            # during the first fc2(e) groups.
            w2s_ = [None] * (EPC * (MT2 // 2))

            def _load_w2(w2i):
                w2 = wtp.tile([P, 2 * KT2, P], MM, tag="w2", name=f"w2_{w2i}")
                nc.sync.dma_start(out=w2, in_=w2l[w2i // (MT2 // 2),
                                                 w2i % (MT2 // 2)])
                w2s_[w2i] = w2

            def fc1_group(e, g, w1, xg, b1s, hs):
                for a in range(2):
                    m = 2 * g + a
                    ps = psp.tile([P, C], F32, tag="ps")
                    if FP8_FC1:
                        # DoubleRow: two k-tiles per matmul via 3D
                        # [128, 2, ...] slices of both operands
                        for k in range(0, KT1, 2):
                            nc.tensor.matmul(
                                ps,
                                lhsT=w1[:, a * KT1 + k:a * KT1 + k + 2, :],
                                rhs=xg[:, k:k + 2, :],
                                perf_mode=mybir.MatmulPerfMode.DoubleRow,
                                start=(k == 0), stop=(k == KT1 - 2))
                    else:
                        for k in range(KT1):
                            nc.tensor.matmul(ps, lhsT=w1[:, a * KT1 + k, :],
                                             rhs=xg[:, k, :],
                                             start=(k == 0),
                                             stop=(k == KT1 - 1))
                    hm = hp.tile([P, C], MM, tag="h")
                    nc.scalar.activation(hm, ps, act, bias=b1s[:, m:m + 1])
                    hs.append(hm)

            def fc2_expert(e, hs, b2s):
                for g in range(MT2 // 2):
                    if g < MT2 // 2 - 1:
                        _load_w2(e * (MT2 // 2) + 1 + g)
                    w2 = w2s_[e * (MT2 // 2) + g]
                    for a in range(2):
                        m = 2 * g + a
                        ps2 = psp.tile([P, C], F32, tag="ps")
                        for k in range(KT2):
                            nc.tensor.matmul(ps2, lhsT=w2[:, a * KT2 + k, :],
                                             rhs=hs[k],
                                             start=(k == 0), stop=(k == KT2 - 1))
                        o1 = op_.tile([P, C], BF16, tag="o1")
                        nc.vector.scalar_tensor_tensor(o1, ps2, b2s[:, m:m + 1],
                                                       wts_sb[:, e, :],
                                                       ALU.add, ALU.mult)
                        (nc.sync if e == EPC - 1 else nc.gpsimd).dma_start(
                            out=ot[e, m], in_=o1)

            # Phase order: fc1(e0), fc1(e1) group 0, fc2(e0), fc1(e1) rest,
            # fc2(e1). The hoisted fc1(e1) group (weights via the idle
            # scalar queue) gives the PE work during the conserved wire
            # deficit at the fc1(e0)->fc2(e0) boundary, where fc2's first
            # weight block cannot have landed yet.
            hs0, hs1 = [], []
            w1_e1g0 = None
            for g in range(MT1 // 2):
                if g == 0:
                    w1 = w1f
                else:
                    w1 = wtp.tile([P, 2 * KT1, P], M1, tag="w1")
                    nc.sync.dma_start(out=w1, in_=w1l[0, g])
                if g == 2 and EPC > 1:
                    # xg1 rides the starved scalar ring: 0.3MB at ~1/4 rate
                    # still lands well before the hoisted group (~19us), and
                    # keeping it out of the sync FIFO pulls w2[e0,0] ~1us
                    # earlier at the fc1->fc2 boundary
                    xg1 = xgp.tile([P, KT1, C], M1, tag="xg")
                    nc.scalar.dma_start(out=xg1, in_=xgm[1])
                    xgs[1] = xg1
                    b1n = smp.tile([P, MT1], F32, tag="b1")
                    nc.scalar.dma_start(out=b1n, in_=b1t[1])
                    b1s_[1] = b1n
                if g == 7 and EPC > 1:
                    # w1(e1)-g0 stays on the sync FIFO right after w1(e0)
                    w1_e1g0 = wtp.tile([P, 2 * KT1, P], M1, tag="w1",
                                       name="w1_e1g0")
                    nc.sync.dma_start(out=w1_e1g0, in_=w1l[1, 0])
                    _load_w2(0)
                fc1_group(0, g, w1, xg0, b1f, hs0)

            if EPC > 1:
                fc1_group(1, 0, w1_e1g0, xgs[1], b1s_[1], hs1)
            fc2_expert(0, hs0, b2s_[0])
            for g in range(1, MT1 // 2):
                w1 = wtp.tile([P, 2 * KT1, P], M1, tag="w1")
                nc.sync.dma_start(out=w1, in_=w1l[1, g])
                if g == 5:
                    _load_w2(MT2 // 2)
                fc1_group(1, g, w1, xgs[1], b1s_[1], hs1)
            fc2_expert(1, hs1, b2s_[1])
    nc.compile()
    return nc


def _get_progs():
    if "router" not in _progs:
        _progs["router"] = _build_router()
        _progs["experts"] = _build_experts()
    return _progs["router"], _progs["experts"]


def _run(nc, in_maps, **kw):
    res = bass_utils.run_bass_kernel_spmd(
        nc, in_maps, core_ids=list(range(NCORES)), **kw)
    kernel.last_results.append(res)
    return res


kernel_last_results = []


def kernel(x, Wr, br, W1, b1, W2, b2, _profile=None):
    x = np.ascontiguousarray(np.asarray(x, dtype=np.float32))
    Wr = np.ascontiguousarray(np.asarray(Wr, dtype=np.float32))
    br = np.asarray(br, dtype=np.float32)
    W1 = np.asarray(W1, dtype=np.float32)
    b1 = np.asarray(b1, dtype=np.float32)
    W2 = np.asarray(W2, dtype=np.float32)
    b2 = np.asarray(b2, dtype=np.float32)

    kernel.last_results = []
    router, experts = _get_progs()
    xt = x.reshape(T, D)

    brc = np.ascontiguousarray(np.broadcast_to(br[None, :], (P, E)))
    in_a = []
    for c in range(NCORES):
        xsT = xt[c * TPC:(c + 1) * TPC].T  # [D, TPC]
        xsT_tb = np.ascontiguousarray(
            xsT.reshape(D, TPC // P, P).transpose(1, 0, 2))  # [TB, D, P]
        in_a.append({"xsT": xsT_tb, "wr": Wr, "brc": brc})
    res_a = _run(router, in_a, **(_profile or {}))
    comb = np.concatenate([r["comb"] for r in res_a.results], axis=0)  # [T, E]

    # Host dispatch: pure gather/layout. Token order within an expert is
    # arbitrary; weights travel with the tokens.
    idxs, cnts = [], []
    for e in range(E):
        idx = np.nonzero(comb[:, e])[0]
        idxs.append(idx)
        cnts.append(len(idx))
    kernel.last_cnts = cnts
    if max(cnts) > C:
        return _kernel_fallback_overflow(xt, comb, W1, b1, W2, b2)

    if USE_BF16:
        import ml_dtypes
        mm_np = ml_dtypes.bfloat16
    else:
        mm_np = np.float32
    if FP8_FC1:
        import ml_dtypes
        m1_np = ml_dtypes.float8_e4m3
    else:
        m1_np = mm_np

    def _tile_w(w, kt, mt):
        # [D_in, D_out] -> [mt/2, P, 2*kt, P]: per-DMA block of two output
        # tiles, partition-major so the transfer is contiguous
        t = w.reshape(kt, P, mt, P).transpose(2, 1, 0, 3)      # [m, p, k, f]
        t = t.reshape(mt // 2, 2, P, kt, P).transpose(0, 2, 1, 3, 4)
        return np.ascontiguousarray(t.reshape(mt // 2, P, 2 * kt, P))

    in_b = []
    for c in range(NCORES):
        xg_stack = np.zeros((EPC, P, D // P, C), np.float32)
        wt_stack = np.zeros((EPC, C), np.float32)
        for j in range(EPC):
            e = EPC * c + j
            idx, cnt = idxs[e], cnts[e]
            gT = xt[idx].T  # [D, cnt]
            xg_stack[j, :, :, :cnt] = gT.reshape(D // P, P, cnt).transpose(1, 0, 2)
            wt_stack[j, :cnt] = comb[idx, e]
        w1c = W1[EPC * c:EPC * (c + 1)].astype(m1_np)  # [EPC, D, H]
        w2c = W2[EPC * c:EPC * (c + 1)].astype(mm_np)  # [EPC, H, D]
        w1l = np.stack([_tile_w(w1c[j], D // P, H // P) for j in range(EPC)])
        w2l = np.stack([_tile_w(w2c[j], H // P, D // P) for j in range(EPC)])
        b1c = np.ascontiguousarray(
            b1[EPC * c:EPC * (c + 1)].reshape(EPC, H // P, P).transpose(0, 2, 1))
        b2c = np.ascontiguousarray(
            b2[EPC * c:EPC * (c + 1)].reshape(EPC, D // P, P).transpose(0, 2, 1))
        wtr = np.ascontiguousarray(
            np.broadcast_to(wt_stack[None, :, :], (P, EPC, C)))
        in_b.append({"xgm": xg_stack.astype(m1_np), "w1l": w1l, "b1t": b1c,
                     "w2l": w2l, "b2t": b2c, "wtr": wtr})
    res_b = _run(experts, in_b, **(_profile or {}))

    # Host combine (all-to-all unshard-reduce): the residual stream starts
    # from x on the token's home shard; each of the token's two expert slots
    # adds w_e * MLP_e(x).
    y = xt.copy()
    for e in range(E):
        c, j = divmod(e, EPC)
        o = res_b.results[c]["ot"][j].reshape(D, C).astype(np.float32)  # [D, C]
        idx, cnt = idxs[e], cnts[e]
        y[idx] += o[:, :cnt].T
    if _profile is not None:
        kernel.last_exec_ns = ((res_a.exec_time_ns or 0),
                               (res_b.exec_time_ns or 0))
    return y.reshape(B, S, D)


def _kernel_fallback_overflow(xt, comb, W1, b1, W2, b2):
    """Capacity-overflow escape hatch (never hit for realistic routing):
    exact dense computation on host."""
    try:
        from scipy.special import erf
    except ImportError:
        import math
        erf = np.vectorize(math.erf, otypes=[np.float32])

    def gelu(v):
        return 0.5 * v * (1.0 + erf(v / np.sqrt(2.0)))

    y = xt.copy()
    for e in range(E):
        idx = np.nonzero(comb[:, e])[0]
        if len(idx) == 0:
            continue
        h = gelu(xt[idx] @ W1[e] + b1[e])
        o = h @ W2[e] + b2[e]
        y[idx] += o * comb[idx, e:e + 1]
    return y.reshape(B, S, D)



# revision 61
# speedup vs baseline: 1.0226x; 1.0226x over previous
"""Top-2 MoE (B=2, S=1024, D=1024, E=16, H=2048) on 8 Trainium2 NeuronCores.

Strategy (expert parallelism, per the sharding hint):
  - Launch A (device): token-sharded router. Each core computes logits for
    its T/8 tokens directly in [token, expert] layout (the x-shard k-tile is
    the stationary operand, Wr streams its 16 columns) as fp32r one-pass
    matmuls — fp22 products keep top-2 selection faithful (validated ~1e-7
    effect) at 1/4 the fp32 PE cost. Top-2 via the DVE max8 instruction,
    then the dense combine matrix comb[t, e] (normalized top-2 softmax
    weights, 0 elsewhere) is written out. Critical path uses two coarse DMAs
    (each dma_start costs ~0.7us of sequencer issue time).
  - Host: all-to-all "dispatch" — pure data movement. Tokens are gathered
    per expert (fixed capacity C per expert) and laid out feature-major;
    expert weights are re-tiled so every device DMA is a contiguous block.
  - Launch B (device): expert shards. Core c owns experts 2c, 2c+1 and runs
    the 2-layer exact-GELU MLP on its gathered tokens in [feature, token]
    layout, so W1/W2 load directly as the matmul stationary operand with no
    transposes. fc1 runs in fp8e4m3 with DoubleRow (two k-tiles per matmul,
    halved W1 traffic; end-to-end rel err 1.75e-2, sim-validated against
    the erf-GELU reference); fc2 stays bf16. All big loads ride the sync
    HWDGE ring in exact PE consumption order (the ring drains FIFO, so
    later loads can never starve the running layer; the scalar ring is
    round-robin-starved to ~1/4 bandwidth and only carries the head loads
    plus tiny biases). Phase order fc1(e0), fc1(e1)-g0, fc2(e0),
    fc1(e1) rest, fc2(e1): the hoisted group fills the conserved wire
    deficit at the fc1->fc2 boundary. The combine weight and fc2 bias are
    applied on device in one fused DVE op; outputs return bf16.
  - Host: all-to-all "combine" — the unshard-reduce. The residual stream
    starts from x on the token's home shard and each token's two expert
    slots are scatter-added into it.

If any expert overflows the capacity C (cannot happen for the reference
routing, which peaks at 282 tokens/expert), a bit-exact numpy fallback
computes the full layer instead.

Both launches warm the PE with dummy matmuls on a memset scratch during
the ~7us Tile preamble + DMA-engine ramp, so the real matmuls start at
2.4 GHz instead of paying the ~3.4us HAM cold window.

Measured on the dev harness: router ~21us + experts ~66-68us = ~87.5-88us
under quiet conditions (baseline as staged: 130us in the same
environment). Shared-HBM contention adds up to +3-7us in bad windows.
"""

import numpy as np

import concourse.bacc as bacc
import concourse.mybir as mybir
from concourse.tile import TileContext
from concourse import bass_utils

F32 = mybir.dt.float32
F32R = mybir.dt.float32r
BF16 = mybir.dt.bfloat16
AF = mybir.ActivationFunctionType
ALU = mybir.AluOpType

USE_BF16 = True  # expert-MLP matmul operand dtype (bf16 vs float32r)
FP8_FC1 = True   # fc1 in fp8e4m3 + DoubleRow (2 k-tiles/matmul); fc2 stays bf16
F8 = mybir.dt.float8e4

B, S, D, E, H = 2, 1024, 1024, 16, 2048
T = B * S
TOP_K = 2
TEMP = 1.0
NCORES = 8
EPC = E // NCORES          # experts per core
TPC = T // NCORES          # router tokens per core
C = 288                    # per-expert token capacity (observed max ~282)
P = 128

_progs = {}


def _build_router():
    """Token-major router: logits land directly as [token, expert] tiles
    (x-shard tile is the stationary operand, Wr streams its 16 columns),
    so no PE transpose / identity is needed and the whole critical path
    (Wr, then per-k x chunks) rides the sync HWDGE ring, which starts
    ~6 us earlier than the scalar ring."""
    nc = bacc.Bacc("TRN2", target_bir_lowering=False, debug=False,
                   num_devices=NCORES)
    TB = TPC // P  # token blocks
    xsT = nc.dram_tensor("xsT", [TB, D, P], F32R, kind="ExternalInput").ap()
    wr = nc.dram_tensor("wr", [D, E], F32R, kind="ExternalInput").ap()
    brc = nc.dram_tensor("brc", [P, E], F32, kind="ExternalInput").ap()
    comb = nc.dram_tensor("comb", [TPC, E], F32, kind="ExternalOutput").ap()

    KT = D // P  # 8 contraction tiles
    with TileContext(nc) as tc:
        with (
            tc.tile_pool(name="const", bufs=1) as const,
            tc.tile_pool(name="sb", bufs=2) as sb,
            tc.tile_pool(name="ps", bufs=2, space="PSUM") as psp,
        ):
            # PE warmup: the real MM phase is only ~2-3us, far shorter than
            # the ~3.4us HAM window, so it would run entirely at 1.2 GHz.
            # Dummy matmuls on a memset scratch fill the preamble/DMA-wait
            # window and un-throttle the PE before the real matmuls start.
            scr = const.tile([P, P], BF16, tag="warm")
            nc.vector.memset(scr, 0.0)
            pw = psp.tile([P, P], F32, tag="warm_ps", bufs=1)
            for _w in range(18):
                nc.tensor.matmul(pw, lhsT=scr, rhs=scr, start=True, stop=True)

            # HWDGE dma_start costs ~0.7 us of sequencer issue time, so the
            # critical path uses as few DMAs as possible, split across the
            # scalar (body starts ~5.8us) and sync (~6.8us) sequencers.
            wr_sb = const.tile([P, KT, E], F32R, tag="wr")
            nc.scalar.dma_start(out=wr_sb,
                                in_=wr.rearrange("(k p) e -> p k e", p=P))
            br_sb = const.tile([P, E], F32, tag="br")
            nc.scalar.dma_start(out=br_sb, in_=brc)

            # tb-major layout (host-prepared): each token block's x rides its
            # own ring, so block 0's matmuls + softmax chain pipeline under
            # block 1's load + matmuls instead of everything waiting for the
            # full x shard
            xs = sb.tile([P, TB, KT, P], F32R, tag="xs")
            for tb in range(TB):
                (nc.sync if tb == 0 else nc.scalar).dma_start(
                    out=xs[:, tb], in_=xsT[tb].rearrange("(k p) t -> p k t", p=P))

            for tb in range(TB):
                ps = psp.tile([P, E], F32, tag="lg")
                for k in range(KT):
                    nc.tensor.matmul(ps, lhsT=xs[:, tb, k, :],
                                     rhs=wr_sb[:, k, :],
                                     start=(k == 0), stop=(k == KT - 1))
                lg = sb.tile([P, E], F32, tag="lg_sb")
                nc.vector.tensor_tensor(lg, ps, br_sb, ALU.add)
                mx = sb.tile([P, 8], F32, tag="mx")
                nc.vector.max(mx, lg)
                negm1 = sb.tile([P, 1], F32, tag="negm1")
                nc.vector.tensor_scalar_mul(negm1, mx[:, 0:1], -1.0 / TEMP)
                s = sb.tile([P, E], F32, tag="s")
                nc.scalar.activation(s, lg, AF.Exp, bias=negm1, scale=1.0 / TEMP)
                e2 = sb.tile([P, 1], F32, tag="e2")
                nc.scalar.activation(e2, mx[:, 1:2], AF.Exp, bias=negm1,
                                     scale=1.0 / TEMP)
                den = sb.tile([P, 1], F32, tag="den")
                nc.vector.tensor_scalar_add(den, e2, 1.0)
                rec = sb.tile([P, 1], F32, tag="rec")
                nc.vector.reciprocal(rec, den)
                mask = sb.tile([P, E], F32, tag="mask")
                nc.vector.tensor_tensor(mask, lg, mx[:, 1:2].to_broadcast([P, E]),
                                        ALU.is_ge)
                cmb = sb.tile([P, E], F32, tag="cmb")
                nc.vector.scalar_tensor_tensor(cmb, s, rec, mask,
                                               ALU.mult, ALU.mult)
                (nc.scalar if tb == 0 else nc.sync).dma_start(
                    out=comb[tb * P:(tb + 1) * P, :], in_=cmb)
    nc.compile()
    return nc


def _build_experts(act=AF.Gelu, bf16=USE_BF16):
    assert EPC == 2, "phase schedule below is written for 2 experts/core"
    nc = bacc.Bacc("TRN2", target_bir_lowering=False, debug=False,
                   num_devices=NCORES)
    MT1 = H // P   # 16 fc1 output tiles
    KT1 = D // P   # 8 fc1 contraction tiles
    MT2 = D // P   # 8 fc2 output tiles
    KT2 = H // P   # 16 fc2 contraction tiles
    MM = BF16 if bf16 else F32R
    M1 = F8 if FP8_FC1 else MM  # fc1 operand dtype

    # weights pre-tiled on host, two output tiles per DMA (>=1 MiB transfers)
    w1l = nc.dram_tensor("w1l", [EPC, MT1 // 2, P, 2 * KT1, P], M1,
                         kind="ExternalInput").ap()
    w2l = nc.dram_tensor("w2l", [EPC, MT2 // 2, P, 2 * KT2, P], MM,
                         kind="ExternalInput").ap()
    xgm = nc.dram_tensor("xgm", [EPC, P, KT1, C], M1,
                         kind="ExternalInput").ap()
    b1t = nc.dram_tensor("b1t", [EPC, P, MT1], F32, kind="ExternalInput").ap()
    b2t = nc.dram_tensor("b2t", [EPC, P, MT2], F32, kind="ExternalInput").ap()
    wtr = nc.dram_tensor("wtr", [P, EPC, C], F32, kind="ExternalInput").ap()
    # combined output in bf16: it is scaled by the (<=1) combine weight and
    # added to the fp32 residual on host, so bf16 rounding here is ~1e-3
    ot = nc.dram_tensor("ot", [EPC, MT2, P, C], BF16, kind="ExternalOutput").ap()

    with TileContext(nc) as tc:
        with (
            tc.tile_pool(name="xg", bufs=2) as xgp,
            tc.tile_pool(name="wt", bufs=8) as wtp,
            tc.tile_pool(name="h", bufs=2 * MT1) as hp,
            tc.tile_pool(name="o", bufs=6) as op_,
            tc.tile_pool(name="small", bufs=2) as smp,
            tc.tile_pool(name="const", bufs=1) as cst,
            tc.tile_pool(name="ps", bufs=7, space="PSUM") as psp,
        ):
            # Critical path to the first MM group (expert 0, m-tile 0): one
            # coarse DMA per ring (each dma_start costs ~0.7us of sequencer
            # issue time, so fewer-but-bigger wins), issued first on the two
            # HWDGE sequencers.
            # PE warmup (see router): fills the ~7-11.5us preamble/DMA-wait
            # window so the first real matmuls run at 2.4 GHz
            scr = cst.tile([P, P], BF16, tag="warm")
            nc.vector.memset(scr, 0.0)
            pw = psp.tile([P, P], F32, tag="warm_ps", bufs=1)
            for _w in range(34):
                nc.tensor.matmul(pw, lhsT=scr, rhs=scr, start=True, stop=True)

            xg0 = xgp.tile([P, KT1, C], M1, tag="xg")
            w1f = wtp.tile([P, 2 * KT1, P], M1, tag="w1")
            nc.sync.dma_start(out=xg0, in_=xgm[0])
            nc.scalar.dma_start(out=w1f, in_=w1l[0, 0])
            b1f = smp.tile([P, MT1], F32, tag="b1")
            nc.scalar.dma_start(out=b1f, in_=b1t[0])

            wts_sb = cst.tile([P, EPC, C], F32, tag="wts")
            nc.gpsimd.dma_start(out=wts_sb, in_=wtr)

            # expert-1 activations/biases are issued from inside expert 0's
            # fc1 loop; fc2 biases ride the gpsimd ring
            xgs, b1s_, b2s_ = {0: xg0}, {0: b1f}, {}
            for e in range(EPC):
                b2s = smp.tile([P, MT2], F32, tag="b2")
                nc.gpsimd.dma_start(out=b2s, in_=b2t[e])
                b2s_[e] = b2s

            # All weight loads ride the sync HWDGE ring, issued in exact PE
            # consumption order — the ring drains FIFO at full width, so
            # later loads can never starve the layer currently running.
            # w2[e] blocks are slotted one-ahead: two late in fc1(e), two
            # during the first fc2(e) groups.
            w2s_ = [None] * (EPC * (MT2 // 2))

            def _load_w2(w2i):
                w2 = wtp.tile([P, 2 * KT2, P], MM, tag="w2", name=f"w2_{w2i}")
                nc.sync.dma_start(out=w2, in_=w2l[w2i // (MT2 // 2),
                                                 w2i % (MT2 // 2)])
                w2s_[w2i] = w2

            def fc1_group(e, g, w1, xg, b1s, hs):
                for a in range(2):
                    m = 2 * g + a
                    ps = psp.tile([P, C], F32, tag="ps")
                    if FP8_FC1:
                        # DoubleRow: two k-tiles per matmul via 3D
                        # [128, 2, ...] slices of both operands
                        for k in range(0, KT1, 2):
                            nc.tensor.matmul(
                                ps,
                                lhsT=w1[:, a * KT1 + k:a * KT1 + k + 2, :],
                                rhs=xg[:, k:k + 2, :],
                                perf_mode=mybir.MatmulPerfMode.DoubleRow,
                                start=(k == 0), stop=(k == KT1 - 2))
                    else:
                        for k in range(KT1):
                            nc.tensor.matmul(ps, lhsT=w1[:, a * KT1 + k, :],
                                             rhs=xg[:, k, :],
                                             start=(k == 0),
                                             stop=(k == KT1 - 1))
                    hm = hp.tile([P, C], MM, tag="h")
                    nc.scalar.activation(hm, ps, act, bias=b1s[:, m:m + 1])
                    hs.append(hm)

            def fc2_expert(e, hs, b2s):
                for g in range(MT2 // 2):
                    if g < MT2 // 2 - 1:
                        _load_w2(e * (MT2 // 2) + 1 + g)
                    w2 = w2s_[e * (MT2 // 2) + g]
                    for a in range(2):
                        m = 2 * g + a
                        ps2 = psp.tile([P, C], F32, tag="ps")
                        for k in range(KT2):
                            nc.tensor.matmul(ps2, lhsT=w2[:, a * KT2 + k, :],
                                             rhs=hs[k],
                                             start=(k == 0), stop=(k == KT2 - 1))
                        o1 = op_.tile([P, C], BF16, tag="o1")
                        nc.vector.scalar_tensor_tensor(o1, ps2, b2s[:, m:m + 1],
                                                       wts_sb[:, e, :],
                                                       ALU.add, ALU.mult)
                        (nc.sync if e == EPC - 1 else nc.gpsimd).dma_start(
                            out=ot[e, m], in_=o1)

            # Phase order: fc1(e0), fc1(e1) group 0, fc2(e0), fc1(e1) rest,
            # fc2(e1). The hoisted fc1(e1) group (weights via the idle
            # scalar queue) gives the PE work during the conserved wire
            # deficit at the fc1(e0)->fc2(e0) boundary, where fc2's first
            # weight block cannot have landed yet.
            hs0, hs1 = [], []
            w1_e1g0 = None
            for g in range(MT1 // 2):
                if g == 0:
                    w1 = w1f
                else:
                    w1 = wtp.tile([P, 2 * KT1, P], M1, tag="w1")
                    nc.sync.dma_start(out=w1, in_=w1l[0, g])
                if g == 2 and EPC > 1:
                    b1n = smp.tile([P, MT1], F32, tag="b1")
                    nc.scalar.dma_start(out=b1n, in_=b1t[1])
                    b1s_[1] = b1n
                if g == 7 and EPC > 1:
                    # hoisted fc1(e1)-g0 inputs go on the sync FIFO right
                    # after w1(e0) — any ring reassignment just moves the
                    # conserved wire deficit to a different PE stall
                    w1_e1g0 = wtp.tile([P, 2 * KT1, P], M1, tag="w1",
                                       name="w1_e1g0")
                    nc.sync.dma_start(out=w1_e1g0, in_=w1l[1, 0])
                    xg1 = xgp.tile([P, KT1, C], M1, tag="xg")
                    nc.sync.dma_start(out=xg1, in_=xgm[1])
                    xgs[1] = xg1
                    _load_w2(0)
                fc1_group(0, g, w1, xg0, b1f, hs0)

            if EPC > 1:
                fc1_group(1, 0, w1_e1g0, xgs[1], b1s_[1], hs1)
            fc2_expert(0, hs0, b2s_[0])
            for g in range(1, MT1 // 2):
                w1 = wtp.tile([P, 2 * KT1, P], M1, tag="w1")
                nc.sync.dma_start(out=w1, in_=w1l[1, g])
                if g == 5:
                    _load_w2(MT2 // 2)
                fc1_group(1, g, w1, xgs[1], b1s_[1], hs1)
            fc2_expert(1, hs1, b2s_[1])
    nc.compile()
    return nc


def _get_progs():
    if "router" not in _progs:
        _progs["router"] = _build_router()
        _progs["experts"] = _build_experts()
    return _progs["router"], _progs["experts"]


def _run(nc, in_maps, **kw):
    res = bass_utils.run_bass_kernel_spmd(
        nc, in_maps, core_ids=list(range(NCORES)), **kw)
    kernel.last_results.append(res)
    return res


kernel_last_results = []


def kernel(x, Wr, br, W1, b1, W2, b2, _profile=None):
    x = np.ascontiguousarray(np.asarray(x, dtype=np.float32))
    Wr = np.ascontiguousarray(np.asarray(Wr, dtype=np.float32))
    br = np.asarray(br, dtype=np.float32)
    W1 = np.asarray(W1, dtype=np.float32)
    b1 = np.asarray(b1, dtype=np.float32)
    W2 = np.asarray(W2, dtype=np.float32)
    b2 = np.asarray(b2, dtype=np.float32)

    kernel.last_results = []
    router, experts = _get_progs()
    xt = x.reshape(T, D)

    brc = np.ascontiguousarray(np.broadcast_to(br[None, :], (P, E)))
    in_a = []
    for c in range(NCORES):
        xsT = xt[c * TPC:(c + 1) * TPC].T  # [D, TPC]
        xsT_tb = np.ascontiguousarray(
            xsT.reshape(D, TPC // P, P).transpose(1, 0, 2))  # [TB, D, P]
        in_a.append({"xsT": xsT_tb, "wr": Wr, "brc": brc})
    res_a = _run(router, in_a, **(_profile or {}))
    comb = np.concatenate([r["comb"] for r in res_a.results], axis=0)  # [T, E]

    # Host dispatch: pure gather/layout. Token order within an expert is
    # arbitrary; weights travel with the tokens.
    idxs, cnts = [], []
    for e in range(E):
        idx = np.nonzero(comb[:, e])[0]
        idxs.append(idx)
        cnts.append(len(idx))
    kernel.last_cnts = cnts
    if max(cnts) > C:
        return _kernel_fallback_overflow(xt, comb, W1, b1, W2, b2)

    if USE_BF16:
        import ml_dtypes
        mm_np = ml_dtypes.bfloat16
    else:
        mm_np = np.float32
    if FP8_FC1:
        import ml_dtypes
        m1_np = ml_dtypes.float8_e4m3
    else:
        m1_np = mm_np

    def _tile_w(w, kt, mt):
        # [D_in, D_out] -> [mt/2, P, 2*kt, P]: per-DMA block of two output
        # tiles, partition-major so the transfer is contiguous
        t = w.reshape(kt, P, mt, P).transpose(2, 1, 0, 3)      # [m, p, k, f]
        t = t.reshape(mt // 2, 2, P, kt, P).transpose(0, 2, 1, 3, 4)
        return np.ascontiguousarray(t.reshape(mt // 2, P, 2 * kt, P))

    in_b = []
    for c in range(NCORES):
        xg_stack = np.zeros((EPC, P, D // P, C), np.float32)
        wt_stack = np.zeros((EPC, C), np.float32)
        for j in range(EPC):
            e = EPC * c + j
            idx, cnt = idxs[e], cnts[e]
            gT = xt[idx].T  # [D, cnt]
            xg_stack[j, :, :, :cnt] = gT.reshape(D // P, P, cnt).transpose(1, 0, 2)
            wt_stack[j, :cnt] = comb[idx, e]
        w1c = W1[EPC * c:EPC * (c + 1)].astype(m1_np)  # [EPC, D, H]
        w2c = W2[EPC * c:EPC * (c + 1)].astype(mm_np)  # [EPC, H, D]
        w1l = np.stack([_tile_w(w1c[j], D // P, H // P) for j in range(EPC)])
        w2l = np.stack([_tile_w(w2c[j], H // P, D // P) for j in range(EPC)])
        b1c = np.ascontiguousarray(
            b1[EPC * c:EPC * (c + 1)].reshape(EPC, H // P, P).transpose(0, 2, 1))
        b2c = np.ascontiguousarray(
            b2[EPC * c:EPC * (c + 1)].reshape(EPC, D // P, P).transpose(0, 2, 1))
        wtr = np.ascontiguousarray(
            np.broadcast_to(wt_stack[None, :, :], (P, EPC, C)))
        in_b.append({"xgm": xg_stack.astype(m1_np), "w1l": w1l, "b1t": b1c,
                     "w2l": w2l, "b2t": b2c, "wtr": wtr})
    res_b = _run(experts, in_b, **(_profile or {}))

    # Host combine (all-to-all unshard-reduce): the residual stream starts
    # from x on the token's home shard; each of the token's two expert slots
    # adds w_e * MLP_e(x).
    y = xt.copy()
    for e in range(E):
        c, j = divmod(e, EPC)
        o = res_b.results[c]["ot"][j].reshape(D, C).astype(np.float32)  # [D, C]
        idx, cnt = idxs[e], cnts[e]
        y[idx] += o[:, :cnt].T
    if _profile is not None:
        kernel.last_exec_ns = ((res_a.exec_time_ns or 0),
                               (res_b.exec_time_ns or 0))
    return y.reshape(B, S, D)


def _kernel_fallback_overflow(xt, comb, W1, b1, W2, b2):
    """Capacity-overflow escape hatch (never hit for realistic routing):
    exact dense computation on host."""
    try:
        from scipy.special import erf
    except ImportError:
        import math
        erf = np.vectorize(math.erf, otypes=[np.float32])

    def gelu(v):
        return 0.5 * v * (1.0 + erf(v / np.sqrt(2.0)))

    y = xt.copy()
    for e in range(E):
        idx = np.nonzero(comb[:, e])[0]
        if len(idx) == 0:
            continue
        h = gelu(xt[idx] @ W1[e] + b1[e])
        o = h @ W2[e] + b2[e]
        y[idx] += o * comb[idx, e:e + 1]
    return y.reshape(B, S, D)



# revision 63
# speedup vs baseline: 1.0383x; 1.0153x over previous
"""Top-2 MoE (B=2, S=1024, D=1024, E=16, H=2048) on 8 Trainium2 NeuronCores.

Strategy (expert parallelism, per the sharding hint):
  - Launch A (device): token-sharded router. Each core computes logits for
    its T/8 tokens directly in [token, expert] layout (the x-shard k-tile is
    the stationary operand, Wr streams its 16 columns) as fp32r one-pass
    matmuls — fp22 products keep top-2 selection faithful (validated ~1e-7
    effect) at 1/4 the fp32 PE cost. Top-2 via the DVE max8 instruction,
    then the dense combine matrix comb[t, e] (normalized top-2 softmax
    weights, 0 elsewhere) is written out. Critical path uses two coarse DMAs
    (each dma_start costs ~0.7us of sequencer issue time).
  - Host: all-to-all "dispatch" — pure data movement. Tokens are gathered
    per expert (fixed capacity C per expert) and laid out feature-major;
    expert weights are re-tiled so every device DMA is a contiguous block.
  - Launch B (device): expert shards. Core c owns experts 2c, 2c+1 and runs
    the 2-layer exact-GELU MLP on its gathered tokens in [feature, token]
    layout, so W1/W2 load directly as the matmul stationary operand with no
    transposes. fc1 runs in fp8e4m3 with DoubleRow (two k-tiles per matmul,
    halved W1 traffic; end-to-end rel err 1.75e-2, sim-validated against
    the erf-GELU reference); fc2 stays bf16. All big loads ride the sync
    HWDGE ring in exact PE consumption order (the ring drains FIFO, so
    later loads can never starve the running layer; the scalar ring is
    round-robin-starved to ~1/4 bandwidth and only carries the head loads
    plus tiny biases). Phase order fc1(e0), fc1(e1)-g0, fc2(e0),
    fc1(e1) rest, fc2(e1): the hoisted group fills the conserved wire
    deficit at the fc1->fc2 boundary. The combine weight and fc2 bias are
    applied on device in one fused DVE op; outputs return bf16.
  - Host: all-to-all "combine" — the unshard-reduce. The residual stream
    starts from x on the token's home shard and each token's two expert
    slots are scatter-added into it.

If any expert overflows the capacity C (cannot happen for the reference
routing, which peaks at 282 tokens/expert), a bit-exact numpy fallback
computes the full layer instead.

Both launches warm the PE with dummy matmuls on a memset scratch during
the ~7us Tile preamble + DMA-engine ramp, so the real matmuls start at
2.4 GHz instead of paying the ~3.4us HAM cold window.

Measured on the dev harness: router ~21us + experts ~66-68us = ~87.5-88us
under quiet conditions (baseline as staged: 130us in the same
environment). Shared-HBM contention adds up to +3-7us in bad windows.
"""

import numpy as np

import concourse.bacc as bacc
import concourse.mybir as mybir
from concourse.tile import TileContext
from concourse import bass_utils

F32 = mybir.dt.float32
F32R = mybir.dt.float32r
BF16 = mybir.dt.bfloat16
AF = mybir.ActivationFunctionType
ALU = mybir.AluOpType

USE_BF16 = True  # expert-MLP matmul operand dtype (bf16 vs float32r)
FP8_FC1 = True   # fc1 in fp8e4m3 + DoubleRow (2 k-tiles/matmul); fc2 stays bf16
F8 = mybir.dt.float8e4

B, S, D, E, H = 2, 1024, 1024, 16, 2048
T = B * S
TOP_K = 2
TEMP = 1.0
NCORES = 8
EPC = E // NCORES          # experts per core
TPC = T // NCORES          # router tokens per core
C = 288                    # per-expert token capacity (observed max ~282)
P = 128

_progs = {}


def _build_router():
    """Token-major router: logits land directly as [token, expert] tiles
    (x-shard tile is the stationary operand, Wr streams its 16 columns),
    so no PE transpose / identity is needed and the whole critical path
    (Wr, then per-k x chunks) rides the sync HWDGE ring, which starts
    ~6 us earlier than the scalar ring."""
    nc = bacc.Bacc("TRN2", target_bir_lowering=False, debug=False,
                   num_devices=NCORES)
    TB = TPC // P  # token blocks
    xsT = nc.dram_tensor("xsT", [TB, D, P], F32R, kind="ExternalInput").ap()
    wr = nc.dram_tensor("wr", [D, E], F32R, kind="ExternalInput").ap()
    brc = nc.dram_tensor("brc", [P, E], F32, kind="ExternalInput").ap()
    comb = nc.dram_tensor("comb", [TPC, E], F32, kind="ExternalOutput").ap()

    KT = D // P  # 8 contraction tiles
    with TileContext(nc) as tc:
        with (
            tc.tile_pool(name="const", bufs=1) as const,
            tc.tile_pool(name="sb", bufs=2) as sb,
            tc.tile_pool(name="ps", bufs=2, space="PSUM") as psp,
        ):
            # PE warmup: the real MM phase is only ~2-3us, far shorter than
            # the ~3.4us HAM window, so it would run entirely at 1.2 GHz.
            # Dummy matmuls on a memset scratch fill the preamble/DMA-wait
            # window and un-throttle the PE before the real matmuls start.
            scr = const.tile([P, P], BF16, tag="warm")
            nc.vector.memset(scr, 0.0)
            pw = psp.tile([P, P], F32, tag="warm_ps", bufs=1)
            for _w in range(18):
                nc.tensor.matmul(pw, lhsT=scr, rhs=scr, start=True, stop=True)

            # HWDGE dma_start costs ~0.7 us of sequencer issue time, so the
            # critical path uses as few DMAs as possible, split across the
            # scalar (body starts ~5.8us) and sync (~6.8us) sequencers.
            wr_sb = const.tile([P, KT, E], F32R, tag="wr")
            nc.scalar.dma_start(out=wr_sb,
                                in_=wr.rearrange("(k p) e -> p k e", p=P))
            br_sb = const.tile([P, E], F32, tag="br")
            nc.scalar.dma_start(out=br_sb, in_=brc)

            # tb-major layout (host-prepared): each token block's x rides its
            # own ring, so block 0's matmuls + softmax chain pipeline under
            # block 1's load + matmuls instead of everything waiting for the
            # full x shard
            xs = sb.tile([P, TB, KT, P], F32R, tag="xs")
            for tb in range(TB):
                (nc.sync if tb == 0 else nc.scalar).dma_start(
                    out=xs[:, tb], in_=xsT[tb].rearrange("(k p) t -> p k t", p=P))

            for tb in range(TB):
                ps = psp.tile([P, E], F32, tag="lg")
                for k in range(KT):
                    nc.tensor.matmul(ps, lhsT=xs[:, tb, k, :],
                                     rhs=wr_sb[:, k, :],
                                     start=(k == 0), stop=(k == KT - 1))
                lg = sb.tile([P, E], F32, tag="lg_sb")
                nc.vector.tensor_tensor(lg, ps, br_sb, ALU.add)
                mx = sb.tile([P, 8], F32, tag="mx")
                nc.vector.max(mx, lg)
                negm1 = sb.tile([P, 1], F32, tag="negm1")
                nc.vector.tensor_scalar_mul(negm1, mx[:, 0:1], -1.0 / TEMP)
                s = sb.tile([P, E], F32, tag="s")
                nc.scalar.activation(s, lg, AF.Exp, bias=negm1, scale=1.0 / TEMP)
                e2 = sb.tile([P, 1], F32, tag="e2")
                nc.scalar.activation(e2, mx[:, 1:2], AF.Exp, bias=negm1,
                                     scale=1.0 / TEMP)
                den = sb.tile([P, 1], F32, tag="den")
                nc.vector.tensor_scalar_add(den, e2, 1.0)
                rec = sb.tile([P, 1], F32, tag="rec")
                nc.vector.reciprocal(rec, den)
                mask = sb.tile([P, E], F32, tag="mask")
                nc.vector.tensor_tensor(mask, lg, mx[:, 1:2].to_broadcast([P, E]),
                                        ALU.is_ge)
                cmb = sb.tile([P, E], F32, tag="cmb")
                nc.vector.scalar_tensor_tensor(cmb, s, rec, mask,
                                               ALU.mult, ALU.mult)
                (nc.scalar if tb == 0 else nc.sync).dma_start(
                    out=comb[tb * P:(tb + 1) * P, :], in_=cmb)
    nc.compile()
    return nc


def _build_experts(act=AF.Gelu, bf16=USE_BF16):
    assert EPC == 2, "phase schedule below is written for 2 experts/core"
    nc = bacc.Bacc("TRN2", target_bir_lowering=False, debug=False,
                   num_devices=NCORES)
    MT1 = H // P   # 16 fc1 output tiles
    KT1 = D // P   # 8 fc1 contraction tiles
    MT2 = D // P   # 8 fc2 output tiles
    KT2 = H // P   # 16 fc2 contraction tiles
    MM = BF16 if bf16 else F32R
    M1 = F8 if FP8_FC1 else MM  # fc1 operand dtype

    # weights pre-tiled on host, two output tiles per DMA (>=1 MiB transfers)
    w1l = nc.dram_tensor("w1l", [EPC, MT1 // 2, P, 2 * KT1, P], M1,
                         kind="ExternalInput").ap()
    w2l = nc.dram_tensor("w2l", [EPC, MT2 // 2, P, 2 * KT2, P], MM,
                         kind="ExternalInput").ap()
    xgm = nc.dram_tensor("xgm", [EPC, P, KT1, C], M1,
                         kind="ExternalInput").ap()
    b1t = nc.dram_tensor("b1t", [EPC, P, MT1], F32, kind="ExternalInput").ap()
    b2t = nc.dram_tensor("b2t", [EPC, P, MT2], F32, kind="ExternalInput").ap()
    wtr = nc.dram_tensor("wtr", [P, EPC, C], F32, kind="ExternalInput").ap()
    # combined output in bf16: it is scaled by the (<=1) combine weight and
    # added to the fp32 residual on host, so bf16 rounding here is ~1e-3
    ot = nc.dram_tensor("ot", [EPC, MT2, P, C], BF16, kind="ExternalOutput").ap()

    with TileContext(nc) as tc:
        with (
            tc.tile_pool(name="xg", bufs=2) as xgp,
            tc.tile_pool(name="wt", bufs=8) as wtp,
            tc.tile_pool(name="h", bufs=2 * MT1) as hp,
            tc.tile_pool(name="o", bufs=6) as op_,
            tc.tile_pool(name="small", bufs=2) as smp,
            tc.tile_pool(name="const", bufs=1) as cst,
            tc.tile_pool(name="ps", bufs=7, space="PSUM") as psp,
        ):
            # Critical path to the first MM group (expert 0, m-tile 0): one
            # coarse DMA per ring (each dma_start costs ~0.7us of sequencer
            # issue time, so fewer-but-bigger wins), issued first on the two
            # HWDGE sequencers.
            # PE warmup (see router): fills the ~7-11.5us preamble/DMA-wait
            # window so the first real matmuls run at 2.4 GHz
            scr = cst.tile([P, P], BF16, tag="warm")
            nc.vector.memset(scr, 0.0)
            pw = psp.tile([P, P], F32, tag="warm_ps", bufs=1)
            for _w in range(34):
                nc.tensor.matmul(pw, lhsT=scr, rhs=scr, start=True, stop=True)

            xg0 = xgp.tile([P, KT1, C], M1, tag="xg")
            w1f = wtp.tile([P, 2 * KT1, P], M1, tag="w1")
            nc.sync.dma_start(out=xg0, in_=xgm[0])
            nc.scalar.dma_start(out=w1f, in_=w1l[0, 0])
            b1f = smp.tile([P, MT1], F32, tag="b1")
            nc.scalar.dma_start(out=b1f, in_=b1t[0])

            wts_sb = cst.tile([P, EPC, C], F32, tag="wts")
            nc.gpsimd.dma_start(out=wts_sb, in_=wtr)

            # expert-1 activations/biases are issued from inside expert 0's
            # fc1 loop; fc2 biases ride the gpsimd ring
            xgs, b1s_, b2s_ = {0: xg0}, {0: b1f}, {}
            for e in range(EPC):
                b2s = smp.tile([P, MT2], F32, tag="b2")
                nc.gpsimd.dma_start(out=b2s, in_=b2t[e])
                b2s_[e] = b2s

            # All weight loads ride the sync HWDGE ring, issued in exact PE
            # consumption order — the ring drains FIFO at full width, so
            # later loads can never starve the layer currently running.
            # w2[e] blocks are slotted one-ahead: two late in fc1(e), two
            # during the first fc2(e) groups.
            w2s_ = [None] * (EPC * (MT2 // 2))

            def _load_w2(w2i, split=False):
                w2 = wtp.tile([P, 2 * KT2, P], MM, tag="w2", name=f"w2_{w2i}")
                e_, g_ = divmod(w2i, MT2 // 2)
                if split:
                    # a-half split: fc2's first psum group starts after
                    # 0.5MB instead of 1MB. Safe ONLY at a queue position
                    # with nothing critical behind it (the extra issue
                    # delays later entries by ~0.7us — this sank v14 when
                    # tried at the g5 slot ahead of w1 g6/g7).
                    nc.sync.dma_start(out=w2[:, :KT2, :],
                                      in_=w2l[e_, g_, :, :KT2, :])
                    nc.sync.dma_start(out=w2[:, KT2:, :],
                                      in_=w2l[e_, g_, :, KT2:, :])
                else:
                    nc.sync.dma_start(out=w2, in_=w2l[e_, g_])
                w2s_[w2i] = w2

            def fc1_group(e, g, w1, xg, b1s, hs):
                for a in range(2):
                    m = 2 * g + a
                    ps = psp.tile([P, C], F32, tag="ps")
                    if FP8_FC1:
                        # DoubleRow: two k-tiles per matmul via 3D
                        # [128, 2, ...] slices of both operands
                        for k in range(0, KT1, 2):
                            nc.tensor.matmul(
                                ps,
                                lhsT=w1[:, a * KT1 + k:a * KT1 + k + 2, :],
                                rhs=xg[:, k:k + 2, :],
                                perf_mode=mybir.MatmulPerfMode.DoubleRow,
                                start=(k == 0), stop=(k == KT1 - 2))
                    else:
                        for k in range(KT1):
                            nc.tensor.matmul(ps, lhsT=w1[:, a * KT1 + k, :],
                                             rhs=xg[:, k, :],
                                             start=(k == 0),
                                             stop=(k == KT1 - 1))
                    hm = hp.tile([P, C], MM, tag="h")
                    nc.scalar.activation(hm, ps, act, bias=b1s[:, m:m + 1])
                    hs.append(hm)

            def fc2_expert(e, hs, b2s):
                for g in range(MT2 // 2):
                    if g < MT2 // 2 - 1:
                        _load_w2(e * (MT2 // 2) + 1 + g)
                    w2 = w2s_[e * (MT2 // 2) + g]
                    for a in range(2):
                        m = 2 * g + a
                        ps2 = psp.tile([P, C], F32, tag="ps")
                        for k in range(KT2):
                            nc.tensor.matmul(ps2, lhsT=w2[:, a * KT2 + k, :],
                                             rhs=hs[k],
                                             start=(k == 0), stop=(k == KT2 - 1))
                        o1 = op_.tile([P, C], BF16, tag="o1")
                        nc.vector.scalar_tensor_tensor(o1, ps2, b2s[:, m:m + 1],
                                                       wts_sb[:, e, :],
                                                       ALU.add, ALU.mult)
                        (nc.sync if e == EPC - 1 else nc.gpsimd).dma_start(
                            out=ot[e, m], in_=o1)

            # Phase order: fc1(e0), fc1(e1) group 0, fc2(e0), fc1(e1) rest,
            # fc2(e1). The hoisted fc1(e1) group (weights via the idle
            # scalar queue) gives the PE work during the conserved wire
            # deficit at the fc1(e0)->fc2(e0) boundary, where fc2's first
            # weight block cannot have landed yet.
            hs0, hs1 = [], []
            w1_e1g0 = None
            for g in range(MT1 // 2):
                if g == 0:
                    w1 = w1f
                else:
                    w1 = wtp.tile([P, 2 * KT1, P], M1, tag="w1")
                    nc.sync.dma_start(out=w1, in_=w1l[0, g])
                if g == 2 and EPC > 1:
                    b1n = smp.tile([P, MT1], F32, tag="b1")
                    nc.scalar.dma_start(out=b1n, in_=b1t[1])
                    b1s_[1] = b1n
                if g == 7 and EPC > 1:
                    # hoisted fc1(e1)-g0 inputs go on the sync FIFO right
                    # after w1(e0) — any ring reassignment just moves the
                    # conserved wire deficit to a different PE stall
                    w1_e1g0 = wtp.tile([P, 2 * KT1, P], M1, tag="w1",
                                       name="w1_e1g0")
                    nc.sync.dma_start(out=w1_e1g0, in_=w1l[1, 0])
                    xg1 = xgp.tile([P, KT1, C], M1, tag="xg")
                    nc.sync.dma_start(out=xg1, in_=xgm[1])
                    xgs[1] = xg1
                    _load_w2(0, split=True)
                fc1_group(0, g, w1, xg0, b1f, hs0)

            if EPC > 1:
                fc1_group(1, 0, w1_e1g0, xgs[1], b1s_[1], hs1)
            fc2_expert(0, hs0, b2s_[0])
            for g in range(1, MT1 // 2):
                w1 = wtp.tile([P, 2 * KT1, P], M1, tag="w1")
                nc.sync.dma_start(out=w1, in_=w1l[1, g])
                if g == 5:
                    _load_w2(MT2 // 2)
                fc1_group(1, g, w1, xgs[1], b1s_[1], hs1)
            fc2_expert(1, hs1, b2s_[1])
    nc.compile()
    return nc


def _get_progs():
    if "router" not in _progs:
        _progs["router"] = _build_router()
        _progs["experts"] = _build_experts()
    return _progs["router"], _progs["experts"]


def _run(nc, in_maps, **kw):
    res = bass_utils.run_bass_kernel_spmd(
        nc, in_maps, core_ids=list(range(NCORES)), **kw)
    kernel.last_results.append(res)
    return res


kernel_last_results = []


def kernel(x, Wr, br, W1, b1, W2, b2, _profile=None):
    x = np.ascontiguousarray(np.asarray(x, dtype=np.float32))
    Wr = np.ascontiguousarray(np.asarray(Wr, dtype=np.float32))
    br = np.asarray(br, dtype=np.float32)
    W1 = np.asarray(W1, dtype=np.float32)
    b1 = np.asarray(b1, dtype=np.float32)
    W2 = np.asarray(W2, dtype=np.float32)
    b2 = np.asarray(b2, dtype=np.float32)

    kernel.last_results = []
    router, experts = _get_progs()
    xt = x.reshape(T, D)

    brc = np.ascontiguousarray(np.broadcast_to(br[None, :], (P, E)))
    in_a = []
    for c in range(NCORES):
        xsT = xt[c * TPC:(c + 1) * TPC].T  # [D, TPC]
        xsT_tb = np.ascontiguousarray(
            xsT.reshape(D, TPC // P, P).transpose(1, 0, 2))  # [TB, D, P]
        in_a.append({"xsT": xsT_tb, "wr": Wr, "brc": brc})
    res_a = _run(router, in_a, **(_profile or {}))
    comb = np.concatenate([r["comb"] for r in res_a.results], axis=0)  # [T, E]

    # Host dispatch: pure gather/layout. Token order within an expert is
    # arbitrary; weights travel with the tokens.
    idxs, cnts = [], []
    for e in range(E):
        idx = np.nonzero(comb[:, e])[0]
        idxs.append(idx)
        cnts.append(len(idx))
    kernel.last_cnts = cnts
    if max(cnts) > C:
        return _kernel_fallback_overflow(xt, comb, W1, b1, W2, b2)

    if USE_BF16:
        import ml_dtypes
        mm_np = ml_dtypes.bfloat16
    else:
        mm_np = np.float32
    if FP8_FC1:
        import ml_dtypes
        m1_np = ml_dtypes.float8_e4m3
    else:
        m1_np = mm_np

    def _tile_w(w, kt, mt):
        # [D_in, D_out] -> [mt/2, P, 2*kt, P]: per-DMA block of two output
        # tiles, partition-major so the transfer is contiguous
        t = w.reshape(kt, P, mt, P).transpose(2, 1, 0, 3)      # [m, p, k, f]
        t = t.reshape(mt // 2, 2, P, kt, P).transpose(0, 2, 1, 3, 4)
        return np.ascontiguousarray(t.reshape(mt // 2, P, 2 * kt, P))

    in_b = []
    for c in range(NCORES):
        xg_stack = np.zeros((EPC, P, D // P, C), np.float32)
        wt_stack = np.zeros((EPC, C), np.float32)
        for j in range(EPC):
            e = EPC * c + j
            idx, cnt = idxs[e], cnts[e]
            gT = xt[idx].T  # [D, cnt]
            xg_stack[j, :, :, :cnt] = gT.reshape(D // P, P, cnt).transpose(1, 0, 2)
            wt_stack[j, :cnt] = comb[idx, e]
        w1c = W1[EPC * c:EPC * (c + 1)].astype(m1_np)  # [EPC, D, H]
        w2c = W2[EPC * c:EPC * (c + 1)].astype(mm_np)  # [EPC, H, D]
        w1l = np.stack([_tile_w(w1c[j], D // P, H // P) for j in range(EPC)])
        w2l = np.stack([_tile_w(w2c[j], H // P, D // P) for j in range(EPC)])
        b1c = np.ascontiguousarray(
            b1[EPC * c:EPC * (c + 1)].reshape(EPC, H // P, P).transpose(0, 2, 1))
        b2c = np.ascontiguousarray(
            b2[EPC * c:EPC * (c + 1)].reshape(EPC, D // P, P).transpose(0, 2, 1))
        wtr = np.ascontiguousarray(
            np.broadcast_to(wt_stack[None, :, :], (P, EPC, C)))
        in_b.append({"xgm": xg_stack.astype(m1_np), "w1l": w1l, "b1t": b1c,
                     "w2l": w2l, "b2t": b2c, "wtr": wtr})
    res_b = _run(experts, in_b, **(_profile or {}))

    # Host combine (all-to-all unshard-reduce): the residual stream starts
    # from x on the token's home shard; each of the token's two expert slots
    # adds w_e * MLP_e(x).
    y = xt.copy()
    for e in range(E):
        c, j = divmod(e, EPC)
        o = res_b.results[c]["ot"][j].reshape(D, C).astype(np.float32)  # [D, C]
        idx, cnt = idxs[e], cnts[e]
        y[idx] += o[:, :cnt].T
    if _profile is not None:
        kernel.last_exec_ns = ((res_a.exec_time_ns or 0),
                               (res_b.exec_time_ns or 0))
    return y.reshape(B, S, D)


def _kernel_fallback_overflow(xt, comb, W1, b1, W2, b2):
    """Capacity-overflow escape hatch (never hit for realistic routing):
    exact dense computation on host."""
    try:
        from scipy.special import erf
    except ImportError:
        import math
        erf = np.vectorize(math.erf, otypes=[np.float32])

    def gelu(v):
        return 0.5 * v * (1.0 + erf(v / np.sqrt(2.0)))

    y = xt.copy()
    for e in range(E):
        idx = np.nonzero(comb[:, e])[0]
        if len(idx) == 0:
            continue
        h = gelu(xt[idx] @ W1[e] + b1[e])
        o = h @ W2[e] + b2[e]
        y[idx] += o * comb[idx, e:e + 1]
    return y.reshape(B, S, D)



# revision 64
# speedup vs baseline: 1.0571x; 1.0181x over previous
"""Top-2 MoE (B=2, S=1024, D=1024, E=16, H=2048) on 8 Trainium2 NeuronCores.

Strategy (expert parallelism, per the sharding hint):
  - Launch A (device): token-sharded router. Each core computes logits for
    its T/8 tokens directly in [token, expert] layout (the x-shard k-tile is
    the stationary operand, Wr streams its 16 columns) as fp32r one-pass
    matmuls — fp22 products keep top-2 selection faithful (validated ~1e-7
    effect) at 1/4 the fp32 PE cost. Top-2 via the DVE max8 instruction,
    then the dense combine matrix comb[t, e] (normalized top-2 softmax
    weights, 0 elsewhere) is written out. Critical path uses two coarse DMAs
    (each dma_start costs ~0.7us of sequencer issue time).
  - Host: all-to-all "dispatch" — pure data movement. Tokens are gathered
    per expert (fixed capacity C per expert) and laid out feature-major;
    expert weights are re-tiled so every device DMA is a contiguous block.
  - Launch B (device): expert shards. Core c owns experts 2c, 2c+1 and runs
    the 2-layer exact-GELU MLP on its gathered tokens in [feature, token]
    layout, so W1/W2 load directly as the matmul stationary operand with no
    transposes. fc1 runs in fp8e4m3 with DoubleRow (two k-tiles per matmul,
    halved W1 traffic; end-to-end rel err 1.75e-2, sim-validated against
    the erf-GELU reference); fc2 stays bf16. All big loads ride the sync
    HWDGE ring in exact PE consumption order (the ring drains FIFO, so
    later loads can never starve the running layer; the scalar ring is
    round-robin-starved to ~1/4 bandwidth and only carries the head loads
    plus tiny biases). Phase order fc1(e0), fc1(e1)-g0, fc2(e0),
    fc1(e1) rest, fc2(e1): the hoisted group fills the conserved wire
    deficit at the fc1->fc2 boundary. The combine weight and fc2 bias are
    applied on device in one fused DVE op; outputs return bf16.
  - Host: all-to-all "combine" — the unshard-reduce. The residual stream
    starts from x on the token's home shard and each token's two expert
    slots are scatter-added into it.

If any expert overflows the capacity C (cannot happen for the reference
routing, which peaks at 282 tokens/expert), a bit-exact numpy fallback
computes the full layer instead.

Both launches warm the PE with dummy matmuls on a memset scratch during
the ~7us Tile preamble + DMA-engine ramp, so the real matmuls start at
2.4 GHz instead of paying the ~3.4us HAM cold window.

Measured on the dev harness: router ~21us + experts ~66-68us = ~87.5-88us
under quiet conditions (baseline as staged: 130us in the same
environment). Shared-HBM contention adds up to +3-7us in bad windows.
"""

import numpy as np

import concourse.bacc as bacc
import concourse.mybir as mybir
from concourse.tile import TileContext
from concourse import bass_utils

F32 = mybir.dt.float32
F32R = mybir.dt.float32r
BF16 = mybir.dt.bfloat16
AF = mybir.ActivationFunctionType
ALU = mybir.AluOpType

USE_BF16 = True  # expert-MLP matmul operand dtype (bf16 vs float32r)
FP8_FC1 = True   # fc1 in fp8e4m3 + DoubleRow (2 k-tiles/matmul); fc2 stays bf16
F8 = mybir.dt.float8e4

B, S, D, E, H = 2, 1024, 1024, 16, 2048
T = B * S
TOP_K = 2
TEMP = 1.0
NCORES = 8
EPC = E // NCORES          # experts per core
TPC = T // NCORES          # router tokens per core
C = 288                    # per-expert token capacity (observed max ~282)
P = 128

_progs = {}


def _build_router():
    """Token-major router: logits land directly as [token, expert] tiles
    (x-shard tile is the stationary operand, Wr streams its 16 columns),
    so no PE transpose / identity is needed and the whole critical path
    (Wr, then per-k x chunks) rides the sync HWDGE ring, which starts
    ~6 us earlier than the scalar ring."""
    nc = bacc.Bacc("TRN2", target_bir_lowering=False, debug=False,
                   num_devices=NCORES)
    TB = TPC // P  # token blocks
    xsT = nc.dram_tensor("xsT", [TB, D, P], F32R, kind="ExternalInput").ap()
    wr = nc.dram_tensor("wr", [D, E], F32R, kind="ExternalInput").ap()
    brc = nc.dram_tensor("brc", [P, E], F32, kind="ExternalInput").ap()
    comb = nc.dram_tensor("comb", [TPC, E], F32, kind="ExternalOutput").ap()

    KT = D // P  # 8 contraction tiles
    with TileContext(nc) as tc:
        with (
            tc.tile_pool(name="const", bufs=1) as const,
            tc.tile_pool(name="sb", bufs=2) as sb,
            tc.tile_pool(name="ps", bufs=2, space="PSUM") as psp,
        ):
            # PE warmup: the real MM phase is only ~2-3us, far shorter than
            # the ~3.4us HAM window, so it would run entirely at 1.2 GHz.
            # Dummy matmuls on a memset scratch fill the preamble/DMA-wait
            # window and un-throttle the PE before the real matmuls start.
            scr = const.tile([P, P], BF16, tag="warm")
            nc.vector.memset(scr, 0.0)
            pw = psp.tile([P, P], F32, tag="warm_ps", bufs=1)
            for _w in range(18):
                nc.tensor.matmul(pw, lhsT=scr, rhs=scr, start=True, stop=True)

            # HWDGE dma_start costs ~0.7 us of sequencer issue time, so the
            # critical path uses as few DMAs as possible, split across the
            # scalar (body starts ~5.8us) and sync (~6.8us) sequencers.
            wr_sb = const.tile([P, KT, E], F32R, tag="wr")
            nc.scalar.dma_start(out=wr_sb,
                                in_=wr.rearrange("(k p) e -> p k e", p=P))
            br_sb = const.tile([P, E], F32, tag="br")
            nc.scalar.dma_start(out=br_sb, in_=brc)

            # tb-major layout (host-prepared): each token block's x rides its
            # own ring, so block 0's matmuls + softmax chain pipeline under
            # block 1's load + matmuls instead of everything waiting for the
            # full x shard
            xs = sb.tile([P, TB, KT, P], F32R, tag="xs")
            for tb in range(TB):
                (nc.sync if tb == 0 else nc.scalar).dma_start(
                    out=xs[:, tb], in_=xsT[tb].rearrange("(k p) t -> p k t", p=P))

            for tb in range(TB):
                ps = psp.tile([P, E], F32, tag="lg")
                for k in range(KT):
                    nc.tensor.matmul(ps, lhsT=xs[:, tb, k, :],
                                     rhs=wr_sb[:, k, :],
                                     start=(k == 0), stop=(k == KT - 1))
                lg = sb.tile([P, E], F32, tag="lg_sb")
                nc.vector.tensor_tensor(lg, ps, br_sb, ALU.add)
                mx = sb.tile([P, 8], F32, tag="mx")
                nc.vector.max(mx, lg)
                negm1 = sb.tile([P, 1], F32, tag="negm1")
                nc.vector.tensor_scalar_mul(negm1, mx[:, 0:1], -1.0 / TEMP)
                s = sb.tile([P, E], F32, tag="s")
                nc.scalar.activation(s, lg, AF.Exp, bias=negm1, scale=1.0 / TEMP)
                e2 = sb.tile([P, 1], F32, tag="e2")
                nc.scalar.activation(e2, mx[:, 1:2], AF.Exp, bias=negm1,
                                     scale=1.0 / TEMP)
                den = sb.tile([P, 1], F32, tag="den")
                nc.vector.tensor_scalar_add(den, e2, 1.0)
                rec = sb.tile([P, 1], F32, tag="rec")
                nc.vector.reciprocal(rec, den)
                mask = sb.tile([P, E], F32, tag="mask")
                nc.vector.tensor_tensor(mask, lg, mx[:, 1:2].to_broadcast([P, E]),
                                        ALU.is_ge)
                cmb = sb.tile([P, E], F32, tag="cmb")
                nc.vector.scalar_tensor_tensor(cmb, s, rec, mask,
                                               ALU.mult, ALU.mult)
                (nc.scalar if tb == 0 else nc.sync).dma_start(
                    out=comb[tb * P:(tb + 1) * P, :], in_=cmb)
    nc.compile()
    return nc


def _build_experts(act=AF.Gelu, bf16=USE_BF16):
    assert EPC == 2, "phase schedule below is written for 2 experts/core"
    nc = bacc.Bacc("TRN2", target_bir_lowering=False, debug=False,
                   num_devices=NCORES)
    MT1 = H // P   # 16 fc1 output tiles
    KT1 = D // P   # 8 fc1 contraction tiles
    MT2 = D // P   # 8 fc2 output tiles
    KT2 = H // P   # 16 fc2 contraction tiles
    MM = BF16 if bf16 else F32R
    M1 = F8 if FP8_FC1 else MM  # fc1 operand dtype

    # weights pre-tiled on host, two output tiles per DMA (>=1 MiB transfers)
    w1l = nc.dram_tensor("w1l", [EPC, MT1 // 2, P, 2 * KT1, P], M1,
                         kind="ExternalInput").ap()
    w2l = nc.dram_tensor("w2l", [EPC, MT2 // 2, P, 2 * KT2, P], MM,
                         kind="ExternalInput").ap()
    xgm = nc.dram_tensor("xgm", [EPC, P, KT1, C], M1,
                         kind="ExternalInput").ap()
    b1t = nc.dram_tensor("b1t", [EPC, P, MT1], F32, kind="ExternalInput").ap()
    b2t = nc.dram_tensor("b2t", [EPC, P, MT2], F32, kind="ExternalInput").ap()
    wtr = nc.dram_tensor("wtr", [P, EPC, C], F32, kind="ExternalInput").ap()
    # combined output in bf16: it is scaled by the (<=1) combine weight and
    # added to the fp32 residual on host, so bf16 rounding here is ~1e-3
    ot = nc.dram_tensor("ot", [EPC, MT2, P, C], BF16, kind="ExternalOutput").ap()

    with TileContext(nc) as tc:
        with (
            tc.tile_pool(name="xg", bufs=2) as xgp,
            tc.tile_pool(name="wt", bufs=8) as wtp,
            tc.tile_pool(name="h", bufs=2 * MT1) as hp,
            tc.tile_pool(name="o", bufs=6) as op_,
            tc.tile_pool(name="small", bufs=2) as smp,
            tc.tile_pool(name="const", bufs=1) as cst,
            tc.tile_pool(name="ps", bufs=7, space="PSUM") as psp,
        ):
            # Critical path to the first MM group (expert 0, m-tile 0): one
            # coarse DMA per ring (each dma_start costs ~0.7us of sequencer
            # issue time, so fewer-but-bigger wins), issued first on the two
            # HWDGE sequencers.
            # PE warmup (see router): fills the ~7-11.5us preamble/DMA-wait
            # window so the first real matmuls run at 2.4 GHz
            scr = cst.tile([P, P], BF16, tag="warm")
            nc.vector.memset(scr, 0.0)
            pw = psp.tile([P, P], F32, tag="warm_ps", bufs=1)
            for _w in range(34):
                nc.tensor.matmul(pw, lhsT=scr, rhs=scr, start=True, stop=True)

            xg0 = xgp.tile([P, KT1, C], M1, tag="xg")
            w1f = wtp.tile([P, 2 * KT1, P], M1, tag="w1")
            nc.sync.dma_start(out=xg0, in_=xgm[0])
            nc.scalar.dma_start(out=w1f, in_=w1l[0, 0])
            b1f = smp.tile([P, MT1], F32, tag="b1")
            nc.scalar.dma_start(out=b1f, in_=b1t[0])

            wts_sb = cst.tile([P, EPC, C], F32, tag="wts")
            nc.gpsimd.dma_start(out=wts_sb, in_=wtr)

            # expert-1 activations/biases are issued from inside expert 0's
            # fc1 loop; fc2 biases ride the gpsimd ring
            xgs, b1s_, b2s_ = {0: xg0}, {0: b1f}, {}
            for e in range(EPC):
                b2s = smp.tile([P, MT2], F32, tag="b2")
                nc.gpsimd.dma_start(out=b2s, in_=b2t[e])
                b2s_[e] = b2s

            # All weight loads ride the sync HWDGE ring, issued in exact PE
            # consumption order — the ring drains FIFO at full width, so
            # later loads can never starve the layer currently running.
            # w2[e] blocks are slotted one-ahead: two late in fc1(e), two
            # during the first fc2(e) groups.
            w2s_ = [None] * (EPC * (MT2 // 2))

            def _load_w2(w2i, split=False):
                w2 = wtp.tile([P, 2 * KT2, P], MM, tag="w2", name=f"w2_{w2i}")
                e_, g_ = divmod(w2i, MT2 // 2)
                if split:
                    # progressive split: fc2's first 8 matmuls need only the
                    # first k-quarter (0.25MB), so the boundary stall shrinks
                    # to a quarter-block transit. Safe ONLY at a queue
                    # position with nothing critical behind it (the extra
                    # issues delay later entries by ~0.7us each — this sank
                    # the same split when tried ahead of w1 g6/g7).
                    h2 = KT2 // 2
                    nc.sync.dma_start(out=w2[:, :h2, :],
                                      in_=w2l[e_, g_, :, :h2, :])
                    nc.sync.dma_start(out=w2[:, h2:KT2, :],
                                      in_=w2l[e_, g_, :, h2:KT2, :])
                    nc.sync.dma_start(out=w2[:, KT2:, :],
                                      in_=w2l[e_, g_, :, KT2:, :])
                else:
                    nc.sync.dma_start(out=w2, in_=w2l[e_, g_])
                w2s_[w2i] = w2

            def fc1_group(e, g, w1, xg, b1s, hs):
                for a in range(2):
                    m = 2 * g + a
                    ps = psp.tile([P, C], F32, tag="ps")
                    if FP8_FC1:
                        # DoubleRow: two k-tiles per matmul via 3D
                        # [128, 2, ...] slices of both operands
                        for k in range(0, KT1, 2):
                            nc.tensor.matmul(
                                ps,
                                lhsT=w1[:, a * KT1 + k:a * KT1 + k + 2, :],
                                rhs=xg[:, k:k + 2, :],
                                perf_mode=mybir.MatmulPerfMode.DoubleRow,
                                start=(k == 0), stop=(k == KT1 - 2))
                    else:
                        for k in range(KT1):
                            nc.tensor.matmul(ps, lhsT=w1[:, a * KT1 + k, :],
                                             rhs=xg[:, k, :],
                                             start=(k == 0),
                                             stop=(k == KT1 - 1))
                    hm = hp.tile([P, C], MM, tag="h")
                    nc.scalar.activation(hm, ps, act, bias=b1s[:, m:m + 1])
                    hs.append(hm)

            def fc2_expert(e, hs, b2s):
                for g in range(MT2 // 2):
                    if g < MT2 // 2 - 1:
                        _load_w2(e * (MT2 // 2) + 1 + g)
                    w2 = w2s_[e * (MT2 // 2) + g]
                    for a in range(2):
                        m = 2 * g + a
                        ps2 = psp.tile([P, C], F32, tag="ps")
                        for k in range(KT2):
                            nc.tensor.matmul(ps2, lhsT=w2[:, a * KT2 + k, :],
                                             rhs=hs[k],
                                             start=(k == 0), stop=(k == KT2 - 1))
                        o1 = op_.tile([P, C], BF16, tag="o1")
                        nc.vector.scalar_tensor_tensor(o1, ps2, b2s[:, m:m + 1],
                                                       wts_sb[:, e, :],
                                                       ALU.add, ALU.mult)
                        (nc.sync if e == EPC - 1 else nc.gpsimd).dma_start(
                            out=ot[e, m], in_=o1)

            # Phase order: fc1(e0), fc1(e1) group 0, fc2(e0), fc1(e1) rest,
            # fc2(e1). The hoisted fc1(e1) group (weights via the idle
            # scalar queue) gives the PE work during the conserved wire
            # deficit at the fc1(e0)->fc2(e0) boundary, where fc2's first
            # weight block cannot have landed yet.
            hs0, hs1 = [], []
            w1_e1g0 = None
            for g in range(MT1 // 2):
                if g == 0:
                    w1 = w1f
                else:
                    w1 = wtp.tile([P, 2 * KT1, P], M1, tag="w1")
                    nc.sync.dma_start(out=w1, in_=w1l[0, g])
                if g == 2 and EPC > 1:
                    b1n = smp.tile([P, MT1], F32, tag="b1")
                    nc.scalar.dma_start(out=b1n, in_=b1t[1])
                    b1s_[1] = b1n
                if g == 7 and EPC > 1:
                    # hoisted fc1(e1)-g0 inputs go on the sync FIFO right
                    # after w1(e0) — any ring reassignment just moves the
                    # conserved wire deficit to a different PE stall
                    w1_e1g0 = wtp.tile([P, 2 * KT1, P], M1, tag="w1",
                                       name="w1_e1g0")
                    nc.sync.dma_start(out=w1_e1g0, in_=w1l[1, 0])
                    xg1 = xgp.tile([P, KT1, C], M1, tag="xg")
                    nc.sync.dma_start(out=xg1, in_=xgm[1])
                    xgs[1] = xg1
                    _load_w2(0, split=True)
                fc1_group(0, g, w1, xg0, b1f, hs0)

            if EPC > 1:
                fc1_group(1, 0, w1_e1g0, xgs[1], b1s_[1], hs1)
            fc2_expert(0, hs0, b2s_[0])
            for g in range(1, MT1 // 2):
                w1 = wtp.tile([P, 2 * KT1, P], M1, tag="w1")
                nc.sync.dma_start(out=w1, in_=w1l[1, g])
                if g == 5:
                    _load_w2(MT2 // 2)
                fc1_group(1, g, w1, xgs[1], b1s_[1], hs1)
            fc2_expert(1, hs1, b2s_[1])
    nc.compile()
    return nc


def _get_progs():
    if "router" not in _progs:
        _progs["router"] = _build_router()
        _progs["experts"] = _build_experts()
    return _progs["router"], _progs["experts"]


def _run(nc, in_maps, **kw):
    res = bass_utils.run_bass_kernel_spmd(
        nc, in_maps, core_ids=list(range(NCORES)), **kw)
    kernel.last_results.append(res)
    return res


kernel_last_results = []


def kernel(x, Wr, br, W1, b1, W2, b2, _profile=None):
    x = np.ascontiguousarray(np.asarray(x, dtype=np.float32))
    Wr = np.ascontiguousarray(np.asarray(Wr, dtype=np.float32))
    br = np.asarray(br, dtype=np.float32)
    W1 = np.asarray(W1, dtype=np.float32)
    b1 = np.asarray(b1, dtype=np.float32)
    W2 = np.asarray(W2, dtype=np.float32)
    b2 = np.asarray(b2, dtype=np.float32)

    kernel.last_results = []
    router, experts = _get_progs()
    xt = x.reshape(T, D)

    brc = np.ascontiguousarray(np.broadcast_to(br[None, :], (P, E)))
    in_a = []
    for c in range(NCORES):
        xsT = xt[c * TPC:(c + 1) * TPC].T  # [D, TPC]
        xsT_tb = np.ascontiguousarray(
            xsT.reshape(D, TPC // P, P).transpose(1, 0, 2))  # [TB, D, P]
        in_a.append({"xsT": xsT_tb, "wr": Wr, "brc": brc})
    res_a = _run(router, in_a, **(_profile or {}))
    comb = np.concatenate([r["comb"] for r in res_a.results], axis=0)  # [T, E]

    # Host dispatch: pure gather/layout. Token order within an expert is
    # arbitrary; weights travel with the tokens.
    idxs, cnts = [], []
    for e in range(E):
        idx = np.nonzero(comb[:, e])[0]
        idxs.append(idx)
        cnts.append(len(idx))
    kernel.last_cnts = cnts
    if max(cnts) > C:
        return _kernel_fallback_overflow(xt, comb, W1, b1, W2, b2)

    if USE_BF16:
        import ml_dtypes
        mm_np = ml_dtypes.bfloat16
    else:
        mm_np = np.float32
    if FP8_FC1:
        import ml_dtypes
        m1_np = ml_dtypes.float8_e4m3
    else:
        m1_np = mm_np

    def _tile_w(w, kt, mt):
        # [D_in, D_out] -> [mt/2, P, 2*kt, P]: per-DMA block of two output
        # tiles, partition-major so the transfer is contiguous
        t = w.reshape(kt, P, mt, P).transpose(2, 1, 0, 3)      # [m, p, k, f]
        t = t.reshape(mt // 2, 2, P, kt, P).transpose(0, 2, 1, 3, 4)
        return np.ascontiguousarray(t.reshape(mt // 2, P, 2 * kt, P))

    in_b = []
    for c in range(NCORES):
        xg_stack = np.zeros((EPC, P, D // P, C), np.float32)
        wt_stack = np.zeros((EPC, C), np.float32)
        for j in range(EPC):
            e = EPC * c + j
            idx, cnt = idxs[e], cnts[e]
            gT = xt[idx].T  # [D, cnt]
            xg_stack[j, :, :, :cnt] = gT.reshape(D // P, P, cnt).transpose(1, 0, 2)
            wt_stack[j, :cnt] = comb[idx, e]
        w1c = W1[EPC * c:EPC * (c + 1)].astype(m1_np)  # [EPC, D, H]
        w2c = W2[EPC * c:EPC * (c + 1)].astype(mm_np)  # [EPC, H, D]
        w1l = np.stack([_tile_w(w1c[j], D // P, H // P) for j in range(EPC)])
        w2l = np.stack([_tile_w(w2c[j], H // P, D // P) for j in range(EPC)])
        b1c = np.ascontiguousarray(
            b1[EPC * c:EPC * (c + 1)].reshape(EPC, H // P, P).transpose(0, 2, 1))
        b2c = np.ascontiguousarray(
            b2[EPC * c:EPC * (c + 1)].reshape(EPC, D // P, P).transpose(0, 2, 1))
        wtr = np.ascontiguousarray(
            np.broadcast_to(wt_stack[None, :, :], (P, EPC, C)))
        in_b.append({"xgm": xg_stack.astype(m1_np), "w1l": w1l, "b1t": b1c,
                     "w2l": w2l, "b2t": b2c, "wtr": wtr})
    res_b = _run(experts, in_b, **(_profile or {}))

    # Host combine (all-to-all unshard-reduce): the residual stream starts
    # from x on the token's home shard; each of the token's two expert slots
    # adds w_e * MLP_e(x).
    y = xt.copy()
    for e in range(E):
        c, j = divmod(e, EPC)
        o = res_b.results[c]["ot"][j].reshape(D, C).astype(np.float32)  # [D, C]
        idx, cnt = idxs[e], cnts[e]
        y[idx] += o[:, :cnt].T
    if _profile is not None:
        kernel.last_exec_ns = ((res_a.exec_time_ns or 0),
                               (res_b.exec_time_ns or 0))
    return y.reshape(B, S, D)


def _kernel_fallback_overflow(xt, comb, W1, b1, W2, b2):
    """Capacity-overflow escape hatch (never hit for realistic routing):
    exact dense computation on host."""
    try:
        from scipy.special import erf
    except ImportError:
        import math
        erf = np.vectorize(math.erf, otypes=[np.float32])

    def gelu(v):
        return 0.5 * v * (1.0 + erf(v / np.sqrt(2.0)))

    y = xt.copy()
    for e in range(E):
        idx = np.nonzero(comb[:, e])[0]
        if len(idx) == 0:
            continue
        h = gelu(xt[idx] @ W1[e] + b1[e])
        o = h @ W2[e] + b2[e]
        y[idx] += o * comb[idx, e:e + 1]
    return y.reshape(B, S, D)

